# revision 2
# baseline (speedup 1.0000x reference)
"""Trainium2 Bass kernel for nn_NetAtom (Behler-Parrinello segment reduce), v5.

Full-input contract: kernel(**inputs) takes the complete (unsharded) numpy
arrays from setup_inputs() and returns the full [2K] output.

v5 strategy: on HW every instruction carries ~100-300ns of fixed overhead
(ldweights, seq decode, semaphores), so v5 minimizes instruction count at
equal work. vs v4:
  - L1 is ONE DoubleRow matmul per h-block per chunk-pair: desc is packed
    as [64, 2, n] (d-halves as the two k-tiles) so the fp8 DoubleRow mode
    contracts all 128 descriptor dims at 2 rows/cycle, free dim = both
    chunks (1024 cols).
  - L2 is ONE DoubleRow matmul per h-block per pair (rhs spans both chunks).
  - Layer-3 / v extraction / quartic softplus / matvec all run per PAIR
    (half the DVE instructions and PSUM ring churn of per-chunk).
  - Everything else as v4: all-fp8, pair-fused ACT tanh (1024-col
    instructions), runtime-fitted quartic softplus in pv (no Ln, no Exp,
    no activation-table switches), KP=1008, fp8 DoubleRow matvec into two
    persistent PSUM accumulators.
  - Host: sum the 8 per-core [2,1000] partials, concat -> [2000].
"""

import contextlib
from collections import deque

import numpy as np
import ml_dtypes

import concourse.mybir as mybir
import concourse.tile as tile
from concourse import bacc
from concourse.bass_utils import run_bass_kernel_spmd

BF = mybir.dt.bfloat16
F8 = mybir.dt.float8e4
F32 = mybir.dt.float32
ACTF = mybir.ActivationFunctionType
ALU = mybir.AluOpType
DR = mybir.MatmulPerfMode.DoubleRow

D = 128        # descriptor size
H = 256        # hidden width
N = 100000     # atoms per species (full)
K = 1000       # structures
NCORES = 8
CHUNK = 512    # atoms per pipeline chunk
NJ = CHUNK // 128          # 128-atom subchunks per chunk
NCH = 25                   # chunks per core per species
NA = NCH * CHUNK           # 12800 atoms per core (padded); 8*12800 = 102400
KP = 1008                  # padded K stride (16B-aligned j-step)
KH = KP // 2               # structure half (one PSUM bank each); 500 real
KR = K // 2                # real structures per half
MV_DRAIN = 1   # matvec pair-units emitted per pipeline slot
SC = 2         # chunks per superchunk DMA (= pair)
L1DR64 = False  # layer-1 via [64, 2, .] DoubleRow

# fp8 weight pack columns: w1 (512) | w2t (512) | w3t padded (2*16).
# With L1DR64 the w1 section holds the [64, 2 dk, 256 h] layout flattened
# per partition (partitions 64:128 zero); otherwise plain [128 d, 256 h].
W1C = 2 * H
W2C = 2 * H
W8COLS = W1C + W2C + 32

# softplus(pv + b3) ~ per-species quartic in pv fitted at runtime on
# [-SP_FIT_B, SP_FIT_B] (max err ~4e-4; |pv| <= ~0.6 in practice), evaluated
# as out = c4*s3 + c0 with s_{k+1} = (s_k + a_k)*pv; the 5 parameters ride
# in bpack columns 6..10 as per-partition scalars.
SP_FIT_B = 2.0


def build_nc(repeat=None, mode='full'):
    nc = bacc.Bacc()

    ins = {}
    for s in (0, 1):
        ins[f"logicL{s}"] = nc.dram_tensor(f"logicL{s}", [128, NCH * NJ * KP],
                                           F8, kind="ExternalInput")
        dshape = [64, 2 * NA] if L1DR64 else [D, NA]
        ins[f"descT{s}"] = nc.dram_tensor(f"descT{s}", dshape, F8,
                                          kind="ExternalInput")
        ins[f"wpack8{s}"] = nc.dram_tensor(f"wpack8{s}", [128, W8COLS], F8,
                                           kind="ExternalInput")
        ins[f"bpack{s}"] = nc.dram_tensor(f"bpack{s}", [128, 12], F32,
                                          kind="ExternalInput")
    out_d = nc.dram_tensor("out", [2, K], F32, kind="ExternalOutput")

    with tile.TileContext(nc) as tc:
        with tc.tile_pool(name="consts", bufs=1) as consts, \
             tc.tile_pool(name="descp", bufs=3) as descp, \
             tc.tile_pool(name="logicp", bufs=8) as logicp, \
             tc.tile_pool(name="hp", bufs=3) as hp, \
             tc.tile_pool(name="vp", bufs=3) as vp, \
             tc.tile_pool(name="outp", bufs=1) as outp, \
             tc.tile_pool(name="ps_mlp", bufs=2, space="PSUM") as ps_mlp, \
             tc.tile_pool(name="ps_v", bufs=1, space="PSUM") as ps_v, \
             tc.tile_pool(name="ps_mv", bufs=1, space="PSUM") as ps_mv:

            _stack = contextlib.ExitStack()
            if repeat:
                _stack.enter_context(tc.For_i(0, repeat, 1))

            # ---- constants: packed weights + biases per species ----
            wp1, wp2, wp3, bp = {}, {}, {}, {}
            for s in (0, 1):
                if L1DR64:
                    wp1[s] = consts.tile([64, 2, H], F8, name=f"wp1_{s}")
                    nc.sync.dma_start(
                        out=wp1[s],
                        in_=ins[f"wpack8{s}"][0:64, 0:W1C]
                            .rearrange("p (k h) -> p k h", k=2))
                else:
                    wp1[s] = consts.tile([128, W1C], F8, name=f"wp1_{s}")
                    nc.sync.dma_start(out=wp1[s],
                                      in_=ins[f"wpack8{s}"][:, 0:W1C])
                wp2[s] = consts.tile([128, 2, H], F8, name=f"wp2_{s}")
                nc.sync.dma_start(
                    out=wp2[s],
                    in_=ins[f"wpack8{s}"][:, W1C:W1C + W2C]
                        .rearrange("p (k h) -> p k h", k=2))
                wp3[s] = consts.tile([128, 2, 16], F8, name=f"wp3_{s}")
                nc.sync.dma_start(
                    out=wp3[s],
                    in_=ins[f"wpack8{s}"][:, W1C + W2C:W1C + W2C + 32]
                        .rearrange("p (k w) -> p k w", k=2))
                bp[s] = consts.tile([128, 12], F32, name=f"bp_{s}")
                nc.sync.dma_start(out=bp[s], in_=ins[f"bpack{s}"][:, :])

            def w1_8(s, ht):
                if L1DR64:           # [64 d, 2 dk, 128 h] fp8 DoubleRow
                    return wp1[s][:, :, ht * 128:(ht + 1) * 128]
                return wp1[s][:, ht * 128:(ht + 1) * 128]

            def w2_8dr(s, ht):       # [128 h1, 2 kk, 128 h2] fp8 DoubleRow
                return wp2[s][:, :, ht * 128:(ht + 1) * 128]

            def w3_8dr(s):           # [128 h2, 2 kk, 2] fp8 DoubleRow
                return wp3[s][:, :, 0:2]

            def bias(s, which, i):   # [128, 1] per-partition
                off = {"b1": 0, "b2": 2, "b3": 4, "sp": 6}[which] + i
                return bp[s][:, off:off + 1]

            # ---- matvec accumulators: [2, KH] x2, live for whole kernel.
            # (each needs its own PSUM bank: interleaving an unrelated
            # accumulation stream into the same bank corrupts it on HW) ----
            pmv = [ps_mv.tile([2, KH], F32, name=f"pmv{h}", tag=f"pmv{h}")
                   for h in (0, 1)]

            # pair-units: chunks (c0, c0+1) per species, plus the odd chunk
            units = []
            for s in (0, 1):
                for c0 in range(0, NCH - 1, 2):
                    units.append((s, (c0, c0 + 1)))
                units.append((s, (NCH - 1,)))
            n_units = len(units)
            mv_emitted = [0]
            last_mv = [None]

            def stage_a(unit):
                """Superchunk DMA + layer 1 (one DR matmul + one fused tanh
                per h-block)."""
                s, cs = unit
                nu = len(cs)
                c0 = cs[0]
                if L1DR64:
                    dt = descp.tile([64, 2, SC * CHUNK], F8, name="dt",
                                    tag="dt")
                    nc.gpsimd.dma_start(
                        out=dt[:, :, :nu * CHUNK],
                        in_=ins[f"descT{s}"]
                            .rearrange("p (k n) -> p k n", k=2)
                            [:, :, c0 * CHUNK:(c0 + nu) * CHUNK])
                else:
                    dt = descp.tile([D, SC * CHUNK], F8, name="dt", tag="dt")
                    nc.gpsimd.dma_start(
                        out=dt[:, :nu * CHUNK],
                        in_=ins[f"descT{s}"][:, c0 * CHUNK:(c0 + nu) * CHUNK])
                lt = logicp.tile([128, SC * NJ, KP], F8, name="lt", tag="lt")
                nc.sync.dma_start(
                    out=lt[:, :nu * NJ, :],
                    in_=ins[f"logicL{s}"][:, c0 * NJ * KP:(c0 + nu) * NJ * KP]
                        .rearrange("p (j k) -> p j k", k=KP),
                )
                if mode == 'dma':
                    return dict(s=s, cs=cs, lt=lt, h1p=None)
                h1p = hp.tile([128, 2, 2, CHUNK], F8, name="h1p", tag="h1p")
                for ht in (0, 1):
                    pp = ps_mlp.tile([128, 2, CHUNK], F32, name="pp",
                                     tag="pp")
                    for i in range(nu):
                        if L1DR64:
                            nc.tensor.matmul(
                                pp[:, i, :], lhsT=w1_8(s, ht),
                                rhs=dt[:, :, i * CHUNK:(i + 1) * CHUNK],
                                start=True, stop=True, perf_mode=DR,
                            )
                        else:
                            nc.tensor.matmul(
                                pp[:, i, :], lhsT=w1_8(s, ht),
                                rhs=dt[:, i * CHUNK:(i + 1) * CHUNK],
                                start=True, stop=True,
                            )
                    nc.scalar.activation(
                        h1p[:, ht, :nu, :], pp[:, :nu, :], ACTF.Tanh,
                        bias=bias(s, "b1", ht), scale=1.0,
                    )
                return dict(s=s, cs=cs, lt=lt, h1p=h1p)

            def stage_b(meta):
                """Layer 2: one DoubleRow matmul + one fused tanh per
                h-block, spanning the pair."""
                s, cs, h1p = meta["s"], meta["cs"], meta["h1p"]
                nu = len(cs)
                h2p = hp.tile([128, 2, 2, CHUNK], F8, name="h2p", tag="h2p")
                for ht in (0, 1):
                    pp = ps_mlp.tile([128, 2, CHUNK], F32, name="pp",
                                     tag="pp")
                    for i in range(nu):
                        nc.tensor.matmul(
                            pp[:, i, :], lhsT=w2_8dr(s, ht),
                            rhs=h1p[:, :, i, :],
                            start=True, stop=True,
                            perf_mode=DR,
                        )
                    nc.scalar.activation(
                        h2p[:, ht, :nu, :], pp[:, :nu, :], ACTF.Tanh,
                        bias=bias(s, "b2", ht), scale=1.0,
                    )
                meta["h2p"] = h2p

            def stage_c(meta):
                """Layer 3 (DoubleRow fp8) + v0 + quartic softplus -> vg,
                for the whole pair."""
                s, h2p = meta["s"], meta["h2p"]
                nu = len(meta["cs"])
                nj = nu * NJ
                pvr = ps_v.tile([128, 2 * SC * NJ], F32, name="pv", tag="pv")
                for i in range(nu):
                    for j in range(NJ):
                        jj = i * NJ + j
                        mm = nc.tensor.matmul(
                            pvr[:, 2 * jj:2 * jj + 2],
                            lhsT=h2p[:, :, i, j * 128:(j + 1) * 128],
                            rhs=w3_8dr(s),
                            start=True, stop=True,
                            perf_mode=DR,
                        )
                        if jj == 0 and last_mv[0] is not None:
                            tile.add_dep_helper(
                                mm.ins, last_mv[0].ins, sync=False,
                                reason="order L3 after matvec burst")

                vg = vp.tile([128, SC * NJ, 16], F8, name="vg", tag="vg",
                             bufs=4)
                pve = pvr[:, 0:2 * nj:2]
                pvo = pvr[:, 1:2 * nj:2]
                nc.vector.tensor_scalar_add(vg[:, :nj, 0], pve,
                                            bias(s, "b3", 0))
                # softplus(pvo + b3) = c4*s3 + c0, s_{k+1} = (s_k + a_k)*pc
                # (pc = SBUF copy of pvo: DVE may read only one PSUM operand)
                pc = vp.tile([128, SC * NJ], F32, name="pc", tag="pc",
                             bufs=3)
                nc.vector.tensor_copy(pc[:, :nj], pvo)
                s1 = vp.tile([128, SC * NJ], F32, name="sv1", tag="sv1",
                             bufs=3)
                nc.vector.scalar_tensor_tensor(
                    s1[:, :nj], pc[:, :nj], bias(s, "sp", 0), pc[:, :nj],
                    ALU.add, ALU.mult)
                s2 = vp.tile([128, SC * NJ], F32, name="sv2", tag="sv2",
                             bufs=3)
                nc.vector.scalar_tensor_tensor(
                    s2[:, :nj], s1[:, :nj], bias(s, "sp", 1), pc[:, :nj],
                    ALU.add, ALU.mult)
                nc.vector.scalar_tensor_tensor(
                    s1[:, :nj], s2[:, :nj], bias(s, "sp", 2), pc[:, :nj],
                    ALU.add, ALU.mult)
                nc.vector.tensor_scalar(
                    vg[:, :nj, 1], s1[:, :nj], bias(s, "sp", 3),
                    bias(s, "sp", 4), ALU.mult, ALU.add)
                return dict(lt=meta["lt"], vg=vg, nj=nj)

            def emit_mv(cmeta):
                if mode == 'nomv':
                    mv_emitted[0] += 1
                    return
                lt, vg, nj = cmeta["lt"], cmeta["vg"], cmeta["nj"]
                first = mv_emitted[0] == 0
                last = mv_emitted[0] == n_units - 1
                for jp in range(0, nj, 2):
                    for h in (0, 1):
                        last_mv[0] = nc.tensor.matmul(
                            pmv[h][:, :],
                            lhsT=vg[:, jp:jp + 2, 0:2],
                            rhs=lt[:, jp:jp + 2, h * KH:(h + 1) * KH],
                            start=(first and jp == 0),
                            stop=(last and jp == nj - 2),
                            perf_mode=DR,
                            skip_group_check=True,
                        )
                mv_emitted[0] += 1

            pending = deque()
            prev_a = None
            prev_b = None
            for ui in range(n_units + 2):
                meta = stage_a(units[ui]) if ui < n_units else None
                if mode == 'dma':
                    continue
                if prev_a is not None:
                    stage_b(prev_a)
                for _ in range(MV_DRAIN):
                    if len(pending) > 2 or (prev_a is None and pending):
                        emit_mv(pending.popleft())
                if prev_b is not None:
                    pending.append(stage_c(prev_b))
                prev_b = prev_a
                prev_a = meta

            while pending:
                emit_mv(pending.popleft())

            # ---- writeback ----
            osb = outp.tile([2, K], F32, name="osb")
            if mode == 'full':
                for h in (0, 1):
                    nc.vector.tensor_copy(osb[:, h * KR:(h + 1) * KR],
                                          pmv[h][:, :KR])
            else:
                nc.vector.memset(osb[:, :], 0.0)
            nc.sync.dma_start(out=out_d[:, :], in_=osb[:, :])
            _stack.close()

    nc.compile()
    return nc


_NC_CACHE = None


def _get_nc():
    global _NC_CACHE
    if _NC_CACHE is None:
        _NC_CACHE = build_nc()
    return _NC_CACHE


def make_in_maps(desc0, desc1, logic0, logic1,
                 W1_0, b1_0, W2_0, b2_0, W3_0, b3_0,
                 W1_1, b1_1, W2_1, b2_1, W3_1, b3_1):
    fp8 = ml_dtypes.float8_e4m3
    NPAD = NCORES * NA

    per_species = {}
    for s, (desc, logic, W1, b1v, W2, b2v, W3, b3v) in enumerate((
            (desc0, logic0, W1_0, b1_0, W2_0, b2_0, W3_0, b3_0),
            (desc1, logic1, W1_1, b1_1, W2_1, b2_1, W3_1, b3_1))):
        descT = np.zeros((D, NPAD), dtype=fp8)
        descT[:, :N] = np.asarray(desc, np.float32).T.astype(fp8)
        if L1DR64:
            # [64, 2 dk, n]: d = dk*64 + p
            descT = np.ascontiguousarray(
                descT.reshape(2, 64, NPAD).transpose(1, 0, 2)
                .reshape(64, 2 * NPAD))
        logicT = np.zeros((NPAD, KP), dtype=fp8)
        lT = np.asarray(logic, np.float32).T.astype(fp8)   # [N, K]
        logicT[:N, 0:KR] = lT[:, 0:KR]
        logicT[:N, KH:KH + KR] = lT[:, KR:K]
        # SBUF stream layout: [core][128, NCH*NJ*KP], chunk c at cols
        # c*NJ*KP, subchunk j contiguous KP cols, partition = atom % 128.
        logicL = (logicT.reshape(NCORES, NCH, NJ, 128, KP)
                  .transpose(0, 3, 1, 2, 4)
                  .reshape(NCORES, 128, NCH * NJ * KP))
        logicL = np.ascontiguousarray(logicL)

        w1t = np.asarray(W1, np.float32).T                   # [128, 256]
        w1cols = np.zeros((128, W1C), np.float32)
        if L1DR64:
            # [64, 2 dk, 256 h] flattened to [64, 512]: d = dk*64 + p
            w1cols[0:64] = (w1t.reshape(2, 64, H).transpose(1, 0, 2)
                            .reshape(64, W1C))
        else:
            w1cols[:, 0:H] = w1t
        w2t = (np.asarray(W2, np.float32).T.reshape(2, 128, H)
               .transpose(1, 0, 2).reshape(128, 2 * H))      # [128, 512]
        w3t = (np.asarray(W3, np.float32).T.reshape(2, 128, 2)
               .transpose(1, 0, 2).reshape(128, 4))          # [128, (2,2)]
        w3pad = np.zeros((128, 2, 16), np.float32)
        w3pad[:, :, 0:2] = w3t.reshape(128, 2, 2)
        wpack8 = np.concatenate([w1cols, w2t, w3pad.reshape(128, 32)],
                                axis=1).astype(fp8)

        # fit softplus(x + b3[1]) ~ quartic on [-SP_FIT_B, SP_FIT_B],
        # reparametrized for the (s + a)*x evaluation chain
        xg = np.linspace(-SP_FIT_B, SP_FIT_B, 2001)
        sh = float(np.asarray(b3v)[1])
        yg = np.log1p(np.exp(-np.abs(xg + sh))) + np.maximum(xg + sh, 0)
        Ag = np.stack([xg**k for k in range(5)], axis=1)
        wg = np.ones_like(xg)
        for _ in range(40):
            cg, *_ = np.linalg.lstsq(Ag * wg[:, None], yg * wg, rcond=None)
            eg = Ag @ cg - yg
            wg *= (1 + 0.6 * np.abs(eg) / np.abs(eg).max()) ** 1.5
            wg /= wg.mean()
        p0, p1, p2, p3, p4 = [float(v) for v in cg]
        spc = np.array([p3 / p4, p2 / p4, p1 / p4, p4, p0], np.float32)

        bpack = np.concatenate([
            np.asarray(b1v, np.float32).reshape(2, 128).T,
            np.asarray(b2v, np.float32).reshape(2, 128).T,
            np.broadcast_to(np.asarray(b3v, np.float32), (128, 2)),
            np.broadcast_to(spc, (128, 5)),
            np.zeros((128, 1), np.float32),
        ], axis=1)
        bpack = np.ascontiguousarray(bpack)

        per_species[s] = dict(descT=descT, logicL=logicL,
                              wpack8=wpack8, bpack=bpack)

    in_maps = []
    for c in range(NCORES):
        m = {}
        for s in (0, 1):
            sp = per_species[s]
            if L1DR64:
                dv = sp["descT"].reshape(64, 2, NPAD)
                m[f"descT{s}"] = np.ascontiguousarray(
                    dv[:, :, c * NA:(c + 1) * NA].reshape(64, 2 * NA))
            else:
                m[f"descT{s}"] = sp["descT"][:, c * NA:(c + 1) * NA]
            m[f"logicL{s}"] = sp["logicL"][c]
            m[f"wpack8{s}"] = sp["wpack8"]
            m[f"bpack{s}"] = sp["bpack"]
        in_maps.append(m)
    return in_maps


def run(in_maps, trace=False, **kwargs):
    nc = _get_nc()
    return run_bass_kernel_spmd(nc, in_maps, core_ids=list(range(NCORES)),
                                trace=trace, **kwargs)


def kernel(**inputs):
    in_maps = make_in_maps(**inputs)
    res = run(in_maps)
    total = np.zeros((2, K), np.float64)
    for r in res.results:
        total += r["out"].astype(np.float64)
    return np.concatenate([total[0], total[1]]).astype(np.float32)
